# revision 23
# baseline (speedup 1.0000x reference)
"""GQA forward (B=2,S=2048,E=2048,H=16,G=4,D=128) on 8 TRN2 NeuronCores.

Sharding: core = (batch b, kv-group g), b=core//4, g=core%4. Each core
computes its group's 4 query heads end-to-end (QKV proj + RoPE + causal
attention + partial out-projection over its 512 Wo rows). Host sums the 4
partial outputs per batch and adds the bias (the unshard step of the
row-parallel out-projection).

Device dataflow (bf16 matmuls, fp32 PSUM):
  - x^T is staged as 4 column-block tiles and DMA'd in priority order
    (wk/cos/sin/mask, xtb0, wq, wv, xtb1, xtb2, wo, xtb3) so the K/Q/V
    projections start ~10us in, overlapping the rest of the input load.
  - Wq/Wk columns are permuted evens-first within each 128-col head block so
    RoPE's pair rotation becomes a fixed +-64-partition offset in the
    [d, seq] layout: out[0:64] = u*cos[0:64] - (u*sin)[64:128], etc.
  - Scores are computed transposed, ST[k,q] = K'^T-chunk x Q'-tile, so the
    exp'd tile PT[k,q] is directly the moving operand of the P@V matmul
    (OT[d,q] += V_chunk.T @ PT) -- no on-chip transpose of P.
  - Causal triangle-skip: on diagonal k-chunks (dm=0..3) the ST/exp/PV/sums
    work is restricted to q-columns >= dm*128; only the first 128-col
    subblock is triangular and gets a single [128,128] 0/1 mask multiply.
  - Softmax has no max-subtraction (scaled scores are bounded ~ +-17 here);
    denominators come from a ones-column matmul accumulated alongside PV.
  - Normalization runs entirely off the PE queue: reciprocal_approx_fast
    (DVE) -> partition_broadcast (GpSimd) -> tensor_mul (DVE), so the
    tensor engine flows from one head's chunks straight into the next.
  - Schedule interleaves outproj(qt) behind kproj/qproj/vproj(qt+1) to keep
    the PE busy while the last head's normalization chain completes.
"""

import numpy as np
import ml_dtypes

B, S, E = 2, 2048, 2048
H, G = 16, 4
D = E // H            # 128 head dim
M = H // G            # 4 heads per group
DQ = M * D            # 512 per-core Q columns
QT = 512              # q tile (moving dim)
KC = 128              # k chunk (partition dim of ST)
NEC = E // 128        # 16 contraction chunks
NQT = S // QT         # 4 q tiles
SCALE = 1.0 / float(np.sqrt(D))

_CACHE = {}


def _build_module():
    import concourse.tile as tile
    import concourse.mybir as mybir
    from concourse import bacc
    from contextlib import ExitStack

    bf = mybir.dt.bfloat16
    f32 = mybir.dt.float32
    AF = mybir.ActivationFunctionType

    nc = bacc.Bacc("TRN2", target_bir_lowering=False, debug=False)

    xT = nc.dram_tensor("xT", [E, S], bf, kind="ExternalInput").ap()
    wq = nc.dram_tensor("wq", [E, DQ], bf, kind="ExternalInput").ap()
    wk = nc.dram_tensor("wk", [E, D], bf, kind="ExternalInput").ap()
    wv = nc.dram_tensor("wv", [E, D], bf, kind="ExternalInput").ap()
    wo = nc.dram_tensor("wo", [DQ, E], bf, kind="ExternalInput").ap()
    cos_d = nc.dram_tensor("cos_t", [D, S], bf, kind="ExternalInput").ap()
    sin_d = nc.dram_tensor("sin_t", [D, S], bf, kind="ExternalInput").ap()
    mask_d = nc.dram_tensor("mask", [KC, KC], bf, kind="ExternalInput").ap()
    out_d = nc.dram_tensor("out", [S, E], bf, kind="ExternalOutput").ap()

    with tile.TileContext(nc) as tc, ExitStack() as ctx:
        singles = ctx.enter_context(tc.tile_pool(name="singles", bufs=1))
        ropep = ctx.enter_context(tc.tile_pool(name="ropep", bufs=2))
        ptp = ctx.enter_context(tc.tile_pool(name="ptp", bufs=4))
        finp = ctx.enter_context(tc.tile_pool(name="finp", bufs=2))
        ystp = ctx.enter_context(tc.tile_pool(name="ystp", bufs=4))
        psA = ctx.enter_context(tc.tile_pool(name="psA", bufs=3, space="PSUM"))
        psB = ctx.enter_context(tc.tile_pool(name="psB", bufs=2, space="PSUM"))
        psS = ctx.enter_context(tc.tile_pool(name="psS", bufs=1, space="PSUM"))
        psO = ctx.enter_context(tc.tile_pool(name="psO", bufs=2, space="PSUM"))

        # ---- resident SBUF tensors ----
        xtb = [singles.tile([128, NEC, QT], bf, name=f"xtb{t}", tag=f"xt{t}")
               for t in range(NQT)]
        wq_s = singles.tile([128, NEC, DQ], bf, tag="wq")
        wk_s = singles.tile([128, NEC, D], bf, tag="wk")
        wv_s = singles.tile([128, NEC, D], bf, tag="wv")
        wo_s = singles.tile([128, M, E], bf, tag="wo")          # [o%128, head, e]
        cos_s = singles.tile([128, S], bf, tag="cos")
        sin_s = singles.tile([128, S], bf, tag="sin")
        mask_s = singles.tile([128, KC], bf, tag="mask")
        kt_s = singles.tile([128, S], bf, tag="kt")             # roped K^T [d, k]
        v_s = singles.tile([128, NEC, D], bf, tag="v")          # V natural [k%128, k//128, d]
        qt_s = singles.tile([128, M, S], bf, tag="qt")          # roped Q^T [d, h, q]
        ot_s = singles.tile([128, M, S], bf, tag="ot")          # normalized O^T [d, h, q]
        ones_sq = singles.tile([128, 128], bf, tag="ones_sq")   # dense sums lhsT
        wz = singles.tile([128, QT], bf, tag="wz")              # warmup rhs

        nc.vector.memset(ones_sq, 1.0)
        nc.vector.memset(wz, 0.0)

        # ---- input DMAs: FEW large dma_starts (each fans out across HW-DGE
        # queues); issue order = priority. xtb0 goes in 4 pieces so kproj(0)'s
        # psum chain can chase the arriving ec-chunks. ----
        def dma_xtb(t, pieces=1):
            n = NEC // pieces
            for j in range(pieces):
                nc.sync.dma_start(
                    out=xtb[t][:, j * n:(j + 1) * n, :],
                    in_=xT[j * n * 128:(j + 1) * n * 128,
                           t * QT:(t + 1) * QT].rearrange("(n p) s -> p n s", p=128))

        # cos/sin/mask can arrive late: rope runs off the PE path and kt_s/
        # qt_s results are not consumed until attn(0), ~15us after kproj.
        nc.sync.dma_start(out=wk_s, in_=wk.rearrange("(n p) d -> p n d", p=128))
        dma_xtb(0, pieces=2)
        nc.sync.dma_start(out=wv_s, in_=wv.rearrange("(n p) d -> p n d", p=128))
        for h in range(M):
            nc.sync.dma_start(
                out=wq_s[:, :, h * D:(h + 1) * D],
                in_=wq[:, h * D:(h + 1) * D].rearrange("(n p) d -> p n d", p=128))
        nc.sync.dma_start(out=cos_s, in_=cos_d)
        nc.sync.dma_start(out=sin_s, in_=sin_d)
        nc.sync.dma_start(out=mask_s, in_=mask_d)
        dma_xtb(1)
        nc.sync.dma_start(out=wo_s, in_=wo.rearrange("(h p) e -> p h e", p=128))
        dma_xtb(2)
        dma_xtb(3)

        # ---- PE warmup: run dummy matmuls while the input DMAs stream so
        # the p-state ramp completes before kproj(0)'s first real matmul ----
        for i in range(24):
            wp = psB.tile([128, QT], f32, name="wp", tag="ot")
            nc.tensor.matmul(wp, lhsT=ones_sq, rhs=wz, start=True, stop=True)

        def rope(psum, pos0, n, out_bf):
            """psum [128, n] (evens-first d-layout) -> roped bf16 out_bf.

            sin_s is SIGNED: rows 0:64 hold -sin, rows 64:128 hold +sin, so
            out = psum*cos + swap_halves(psum)*sin_s. The half-swap happens in
            the PSUM-side read of the multiply (mixed PSUM+SBUF operands may
            have different base partitions; SBUF+SBUF may not)."""
            mc = ropep.tile([128, QT], f32, tag="mc")
            ms = ropep.tile([128, QT], f32, tag="ms")
            nc.vector.tensor_mul(mc[:, :n], psum, cos_s[:, pos0:pos0 + n])
            nc.vector.tensor_mul(ms[0:64, :n], psum[64:128], sin_s[0:64, pos0:pos0 + n])
            nc.vector.tensor_mul(ms[64:128, :n], psum[0:64], sin_s[64:128, pos0:pos0 + n])
            nc.vector.tensor_add(out_bf, mc[:, :n], ms[:, :n])

        def kproj(t):
            k0 = t * QT
            kp = psA.tile([128, QT], f32, tag="mm")
            for ec in range(NEC):
                nc.tensor.matmul(kp, lhsT=wk_s[:, ec, :], rhs=xtb[t][:, ec, :],
                                 start=(ec == 0), stop=(ec == NEC - 1))
            rope(kp, k0, QT, kt_s[:, k0:k0 + QT])

        def vproj(t):
            for j in range(QT // 128):
                kc = t * (QT // 128) + j
                vp = psA.tile([128, D], f32, tag="mm")
                for ec in range(NEC):
                    nc.tensor.matmul(vp, lhsT=xtb[t][:, ec, j * 128:(j + 1) * 128],
                                     rhs=wv_s[:, ec, :],
                                     start=(ec == 0), stop=(ec == NEC - 1))
                nc.scalar.copy(v_s[:, kc, :], vp)

        def qproj(qt):
            q0 = qt * QT
            for h in range(M):
                qp = psA.tile([128, QT], f32, tag="mm")
                for ec in range(NEC):
                    nc.tensor.matmul(qp, lhsT=wq_s[:, ec, h * D:(h + 1) * D],
                                     rhs=xtb[qt][:, ec, :],
                                     start=(ec == 0), stop=(ec == NEC - 1))
                rope(qp, q0, QT, qt_s[:, h, q0:q0 + QT])

        def attn(qt, interleave=None):
            """interleave: emitted after head 0's first chunk, so that work
            fills the PE while the first ST->exp chain warms up."""
            q0 = qt * QT
            nkc = (q0 + QT) // KC
            for h in range(M):
                otp = psB.tile([128, QT], f32, tag="ot")
                smp = psS.tile([128, QT], f32, tag="sums")
                for kc in range(nkc):
                    dm = kc - qt * (QT // KC)
                    off = dm * 128 if dm > 0 else 0
                    stp = psA.tile([128, QT], f32, tag="mm")
                    nc.tensor.matmul(stp[:, off:], lhsT=kt_s[:, kc * KC:(kc + 1) * KC],
                                     rhs=qt_s[:, h, q0 + off:q0 + QT],
                                     start=True, stop=True)
                    pt = ptp.tile([128, QT], bf, tag="pt")
                    nc.scalar.activation(pt[:, off:], stp[:, off:], AF.Exp, scale=SCALE)
                    if dm >= 0:
                        nc.vector.tensor_mul(pt[:, off:off + 128],
                                             pt[:, off:off + 128], mask_s)
                    nc.tensor.matmul(otp[:, off:], lhsT=v_s[:, kc, :], rhs=pt[:, off:],
                                     start=(kc == 0), stop=(kc == nkc - 1))
                    nc.tensor.matmul(smp[:, off:], lhsT=ones_sq, rhs=pt[:, off:],
                                     start=(kc == 0), stop=(kc == nkc - 1))
                    if h == 0 and kc == 0 and interleave is not None:
                        interleave()
                        interleave = None
                # normalization, entirely off the PE queue: the dense-ones
                # sums matmul already left den broadcast across partitions
                rec = finp.tile([128, QT], f32, tag="rec")
                nc.vector.reciprocal_approx_fast(rec, smp)
                nc.vector.tensor_mul(ot_s[:, h, q0:q0 + QT], otp, rec)

        def outproj(qt):
            q0 = qt * QT
            for sc in range(QT // 128):
                s0 = q0 + sc * 128
                for et in range(E // QT):
                    yp = psO.tile([128, QT], f32, tag="yp")
                    for h in range(M):
                        nc.tensor.matmul(yp, lhsT=ot_s[:, h, s0:s0 + 128],
                                         rhs=wo_s[:, h, et * QT:(et + 1) * QT],
                                         start=(h == 0), stop=(h == M - 1))
                    yst = ystp.tile([128, QT], bf, tag="yst")
                    nc.scalar.copy(yst, yp)
                    nc.sync.dma_start(
                        out=out_d[s0:s0 + 128, et * QT:(et + 1) * QT], in_=yst)

        # ---- schedule: pipeline outproj(qt) behind proj(qt+1) ----
        kproj(0); vproj(0); qproj(0); attn(0)
        for qt in range(1, NQT):
            kproj(qt); vproj(qt); qproj(qt)
            outproj(qt - 1)
            attn(qt)
        outproj(NQT - 1)

    nc.compile()
    return nc


def get_module():
    if "nc" not in _CACHE:
        _CACHE["nc"] = _build_module()
    return _CACHE["nc"]


def host_prep(x, Wq, Wk, Wv):
    """Build the per-core device input arrays (bf16)."""
    bf = ml_dtypes.bfloat16
    x = np.ascontiguousarray(np.asarray(x, np.float32))

    def perm_cols(W):
        W = np.asarray(W, np.float32).copy()
        for h0 in range(0, W.shape[1], D):
            blk = W[:, h0:h0 + D]
            W[:, h0:h0 + D] = np.concatenate([blk[:, ::2], blk[:, 1::2]], 1)
        return W

    Wq_p = perm_cols(Wq).astype(bf)
    Wk_p = perm_cols(Wk).astype(bf)
    Wv_b = np.asarray(Wv, np.float32).astype(bf)

    inv = 1000.0 ** (-2.0 * np.arange(D // 2, dtype=np.float32) / D)
    ang = np.arange(S, dtype=np.float32)[:, None] * inv[None, :]
    cos_e = np.cos(ang).T
    sin_e = np.sin(ang).T
    cos_t = np.ascontiguousarray(np.concatenate([cos_e, cos_e], 0).astype(bf))
    # signed sin table: rows 0:64 = -sin (for the "even - odd*sin" half),
    # rows 64:128 = +sin (see kernel rope())
    sin_t = np.ascontiguousarray(np.concatenate([-sin_e, sin_e], 0).astype(bf))

    j = np.arange(KC)[None, :]
    p = np.arange(KC)[:, None]
    mask = np.ascontiguousarray((j - p >= 0).astype(bf))

    xT_b = [np.ascontiguousarray(x[b].T).astype(bf) for b in range(B)]
    return xT_b, Wq_p, Wk_p, Wv_b, cos_t, sin_t, mask


def _ensure_ntff_hook():
    """The agent image's `antenv` lacks `axon_hooks`, so trn_boot silently
    skipped registering the NTFF profile hook. Recreate the registry module
    and register the ctypes-based hook so trace=True works under axon."""
    import sys
    import types
    try:
        from antenv import axon_hooks  # noqa: F401
        return
    except ImportError:
        pass
    import antenv
    mod = types.ModuleType("antenv.axon_hooks")
    _h = [None]
    mod.set_axon_ntff_profile_hook = lambda h: _h.__setitem__(0, h)
    mod.get_axon_ntff_profile_hook = lambda: _h[0]
    sys.modules["antenv.axon_hooks"] = mod
    antenv.axon_hooks = mod
    try:
        from trn_agent_boot.trn_boot import _ntff_profile_via_ctypes
        hook = _ntff_profile_via_ctypes("/opt/axon/libaxon_pjrt.so")
        mod.set_axon_ntff_profile_hook(hook)
    except Exception as e:  # degrade to no-trace
        print("ntff hook registration failed:", e)


def run(inputs, trace=False, trace_cores=None):
    from concourse import bass_utils
    if trace:
        _ensure_ntff_hook()

    x = np.asarray(inputs["x"], np.float32)
    Wo = np.asarray(inputs["Wo"], np.float32)
    bo = np.asarray(inputs["bo"], np.float32)
    bf = ml_dtypes.bfloat16

    xT_b, Wq_p, Wk_p, Wv_b, cos_t, sin_t, mask = host_prep(
        x, inputs["Wq"], inputs["Wk"], inputs["Wv"])

    in_maps = []
    for core in range(8):
        b, g = divmod(core, 4)
        in_maps.append(dict(
            xT=xT_b[b],
            wq=np.ascontiguousarray(Wq_p[:, g * DQ:(g + 1) * DQ]),
            wk=np.ascontiguousarray(Wk_p[:, g * D:(g + 1) * D]),
            wv=np.ascontiguousarray(Wv_b[:, g * D:(g + 1) * D]),
            wo=np.ascontiguousarray(Wo[g * DQ:(g + 1) * DQ, :].astype(bf)),
            cos_t=cos_t, sin_t=sin_t, mask=mask,
        ))

    nc = get_module()
    kw = {}
    if trace:
        kw = dict(trace=True,
                  trace_cores=trace_cores if trace_cores is not None else [0])
    res = bass_utils.run_bass_kernel_spmd(nc, in_maps, core_ids=list(range(8)), **kw)

    out = np.empty((B, S, E), np.float32)
    for b in range(B):
        acc = np.zeros((S, E), np.float32)
        for g in range(G):
            acc += np.asarray(res.results[4 * b + g]["out"], dtype=np.float32)
        out[b] = acc + bo[None, :]
    return out, res


def kernel(**inputs):
    out, _ = run(inputs, trace=False)
    return out


# revision 24
# speedup vs baseline: 1.0023x; 1.0023x over previous
"""GQA forward (B=2,S=2048,E=2048,H=16,G=4,D=128) on 8 TRN2 NeuronCores.

Sharding: core = (batch b, kv-group g), b=core//4, g=core%4. Each core
computes its group's 4 query heads end-to-end (QKV proj + RoPE + causal
attention + partial out-projection over its 512 Wo rows). Host sums the 4
partial outputs per batch and adds the bias (the unshard step of the
row-parallel out-projection).

Device dataflow (bf16 matmuls, fp32 PSUM):
  - x^T is staged as 4 column-block tiles and DMA'd in priority order
    (wk/cos/sin/mask, xtb0, wq, wv, xtb1, xtb2, wo, xtb3) so the K/Q/V
    projections start ~10us in, overlapping the rest of the input load.
  - Wq/Wk columns are permuted evens-first within each 128-col head block so
    RoPE's pair rotation becomes a fixed +-64-partition offset in the
    [d, seq] layout: out[0:64] = u*cos[0:64] - (u*sin)[64:128], etc.
  - Scores are computed transposed, ST[k,q] = K'^T-chunk x Q'-tile, so the
    exp'd tile PT[k,q] is directly the moving operand of the P@V matmul
    (OT[d,q] += V_chunk.T @ PT) -- no on-chip transpose of P.
  - Causal triangle-skip: on diagonal k-chunks (dm=0..3) the ST/exp/PV/sums
    work is restricted to q-columns >= dm*128; only the first 128-col
    subblock is triangular and gets a single [128,128] 0/1 mask multiply.
  - Softmax has no max-subtraction (scaled scores are bounded ~ +-17 here);
    denominators come from a ones-column matmul accumulated alongside PV.
  - Normalization runs entirely off the PE queue: reciprocal_approx_fast
    (DVE) -> partition_broadcast (GpSimd) -> tensor_mul (DVE), so the
    tensor engine flows from one head's chunks straight into the next.
  - Schedule interleaves outproj(qt) behind kproj/qproj/vproj(qt+1) to keep
    the PE busy while the last head's normalization chain completes.
"""

import numpy as np
import ml_dtypes

B, S, E = 2, 2048, 2048
H, G = 16, 4
D = E // H            # 128 head dim
M = H // G            # 4 heads per group
DQ = M * D            # 512 per-core Q columns
QT = 512              # q tile (moving dim)
KC = 128              # k chunk (partition dim of ST)
NEC = E // 128        # 16 contraction chunks
NQT = S // QT         # 4 q tiles
SCALE = 1.0 / float(np.sqrt(D))

_CACHE = {}


def _build_module():
    import concourse.tile as tile
    import concourse.mybir as mybir
    from concourse import bacc
    from contextlib import ExitStack

    bf = mybir.dt.bfloat16
    f32 = mybir.dt.float32
    AF = mybir.ActivationFunctionType

    nc = bacc.Bacc("TRN2", target_bir_lowering=False, debug=False)

    xT = nc.dram_tensor("xT", [E, S], bf, kind="ExternalInput").ap()
    wq = nc.dram_tensor("wq", [E, DQ], bf, kind="ExternalInput").ap()
    wk = nc.dram_tensor("wk", [E, D], bf, kind="ExternalInput").ap()
    wv = nc.dram_tensor("wv", [E, D], bf, kind="ExternalInput").ap()
    wo = nc.dram_tensor("wo", [DQ, E], bf, kind="ExternalInput").ap()
    cos_d = nc.dram_tensor("cos_t", [D, S], bf, kind="ExternalInput").ap()
    sin_d = nc.dram_tensor("sin_t", [D, S], bf, kind="ExternalInput").ap()
    mask_d = nc.dram_tensor("mask", [KC, KC], bf, kind="ExternalInput").ap()
    out_d = nc.dram_tensor("out", [S, E], bf, kind="ExternalOutput").ap()

    with tile.TileContext(nc) as tc, ExitStack() as ctx:
        singles = ctx.enter_context(tc.tile_pool(name="singles", bufs=1))
        ropep = ctx.enter_context(tc.tile_pool(name="ropep", bufs=2))
        ptp = ctx.enter_context(tc.tile_pool(name="ptp", bufs=4))
        finp = ctx.enter_context(tc.tile_pool(name="finp", bufs=2))
        ystp = ctx.enter_context(tc.tile_pool(name="ystp", bufs=4))
        psA = ctx.enter_context(tc.tile_pool(name="psA", bufs=3, space="PSUM"))
        psB = ctx.enter_context(tc.tile_pool(name="psB", bufs=2, space="PSUM"))
        psS = ctx.enter_context(tc.tile_pool(name="psS", bufs=1, space="PSUM"))
        psO = ctx.enter_context(tc.tile_pool(name="psO", bufs=2, space="PSUM"))

        # ---- resident SBUF tensors ----
        xtb = [singles.tile([128, NEC, QT], bf, name=f"xtb{t}", tag=f"xt{t}")
               for t in range(NQT)]
        wq_s = singles.tile([128, NEC, DQ], bf, tag="wq")
        wk_s = singles.tile([128, NEC, D], bf, tag="wk")
        wv_s = singles.tile([128, NEC, D], bf, tag="wv")
        wo_s = singles.tile([128, M, E], bf, tag="wo")          # [o%128, head, e]
        cos_s = singles.tile([128, S], bf, tag="cos")
        sin_s = singles.tile([128, S], bf, tag="sin")
        mask_s = singles.tile([128, KC], bf, tag="mask")
        kt_s = singles.tile([128, S], bf, tag="kt")             # roped K^T [d, k]
        v_s = singles.tile([128, NEC, D], bf, tag="v")          # V natural [k%128, k//128, d]
        qt_s = singles.tile([128, M, S], bf, tag="qt")          # roped Q^T [d, h, q]
        ot_s = singles.tile([128, M, S], bf, tag="ot")          # normalized O^T [d, h, q]
        ones_sq = singles.tile([128, 128], bf, tag="ones_sq")   # dense sums lhsT
        wz = singles.tile([128, QT], bf, tag="wz")              # warmup rhs

        nc.vector.memset(ones_sq, 1.0)
        nc.vector.memset(wz, 0.0)

        # ---- input DMAs: FEW large dma_starts (each fans out across HW-DGE
        # queues); issue order = priority. xtb0 goes in 4 pieces so kproj(0)'s
        # psum chain can chase the arriving ec-chunks. ----
        def dma_xtb(t, pieces=1):
            n = NEC // pieces
            for j in range(pieces):
                nc.sync.dma_start(
                    out=xtb[t][:, j * n:(j + 1) * n, :],
                    in_=xT[j * n * 128:(j + 1) * n * 128,
                           t * QT:(t + 1) * QT].rearrange("(n p) s -> p n s", p=128))

        nc.sync.dma_start(out=wk_s, in_=wk.rearrange("(n p) d -> p n d", p=128))
        nc.sync.dma_start(out=cos_s, in_=cos_d)
        nc.sync.dma_start(out=sin_s, in_=sin_d)
        dma_xtb(0, pieces=2)
        nc.sync.dma_start(out=mask_s, in_=mask_d)
        nc.sync.dma_start(out=wv_s, in_=wv.rearrange("(n p) d -> p n d", p=128))
        for h in range(M):
            nc.sync.dma_start(
                out=wq_s[:, :, h * D:(h + 1) * D],
                in_=wq[:, h * D:(h + 1) * D].rearrange("(n p) d -> p n d", p=128))
        dma_xtb(1)
        nc.sync.dma_start(out=wo_s, in_=wo.rearrange("(h p) e -> p h e", p=128))
        dma_xtb(2)
        dma_xtb(3)

        # ---- PE warmup: run dummy matmuls while the input DMAs stream so
        # the p-state ramp completes before kproj(0)'s first real matmul ----
        for i in range(44):
            wp = psB.tile([128, QT], f32, name="wp", tag="ot")
            nc.tensor.matmul(wp, lhsT=ones_sq, rhs=wz, start=True, stop=True)

        def rope(psum, pos0, n, out_bf):
            """psum [128, n] (evens-first d-layout) -> roped bf16 out_bf.

            sin_s is SIGNED: rows 0:64 hold -sin, rows 64:128 hold +sin, so
            out = psum*cos + swap_halves(psum)*sin_s. The half-swap happens in
            the PSUM-side read of the multiply (mixed PSUM+SBUF operands may
            have different base partitions; SBUF+SBUF may not)."""
            mc = ropep.tile([128, QT], f32, tag="mc")
            ms = ropep.tile([128, QT], f32, tag="ms")
            nc.vector.tensor_mul(mc[:, :n], psum, cos_s[:, pos0:pos0 + n])
            nc.vector.tensor_mul(ms[0:64, :n], psum[64:128], sin_s[0:64, pos0:pos0 + n])
            nc.vector.tensor_mul(ms[64:128, :n], psum[0:64], sin_s[64:128, pos0:pos0 + n])
            nc.vector.tensor_add(out_bf, mc[:, :n], ms[:, :n])

        def kproj(t):
            k0 = t * QT
            kp = psA.tile([128, QT], f32, tag="mm")
            for ec in range(NEC):
                nc.tensor.matmul(kp, lhsT=wk_s[:, ec, :], rhs=xtb[t][:, ec, :],
                                 start=(ec == 0), stop=(ec == NEC - 1))
            rope(kp, k0, QT, kt_s[:, k0:k0 + QT])

        def vproj(t):
            for j in range(QT // 128):
                kc = t * (QT // 128) + j
                vp = psA.tile([128, D], f32, tag="mm")
                for ec in range(NEC):
                    nc.tensor.matmul(vp, lhsT=xtb[t][:, ec, j * 128:(j + 1) * 128],
                                     rhs=wv_s[:, ec, :],
                                     start=(ec == 0), stop=(ec == NEC - 1))
                nc.scalar.copy(v_s[:, kc, :], vp)

        def qproj(qt):
            q0 = qt * QT
            for h in range(M):
                qp = psA.tile([128, QT], f32, tag="mm")
                for ec in range(NEC):
                    nc.tensor.matmul(qp, lhsT=wq_s[:, ec, h * D:(h + 1) * D],
                                     rhs=xtb[qt][:, ec, :],
                                     start=(ec == 0), stop=(ec == NEC - 1))
                rope(qp, q0, QT, qt_s[:, h, q0:q0 + QT])

        def attn(qt, interleave=None):
            """interleave: emitted after head 0's first chunk, so that work
            fills the PE while the first ST->exp chain warms up."""
            q0 = qt * QT
            nkc = (q0 + QT) // KC
            for h in range(M):
                otp = psB.tile([128, QT], f32, tag="ot")
                smp = psS.tile([128, QT], f32, tag="sums")
                for kc in range(nkc):
                    dm = kc - qt * (QT // KC)
                    off = dm * 128 if dm > 0 else 0
                    stp = psA.tile([128, QT], f32, tag="mm")
                    nc.tensor.matmul(stp[:, off:], lhsT=kt_s[:, kc * KC:(kc + 1) * KC],
                                     rhs=qt_s[:, h, q0 + off:q0 + QT],
                                     start=True, stop=True)
                    pt = ptp.tile([128, QT], bf, tag="pt")
                    nc.scalar.activation(pt[:, off:], stp[:, off:], AF.Exp, scale=SCALE)
                    if dm >= 0:
                        nc.vector.tensor_mul(pt[:, off:off + 128],
                                             pt[:, off:off + 128], mask_s)
                    nc.tensor.matmul(otp[:, off:], lhsT=v_s[:, kc, :], rhs=pt[:, off:],
                                     start=(kc == 0), stop=(kc == nkc - 1))
                    nc.tensor.matmul(smp[:, off:], lhsT=ones_sq, rhs=pt[:, off:],
                                     start=(kc == 0), stop=(kc == nkc - 1))
                    if h == 0 and kc == 0 and interleave is not None:
                        interleave()
                        interleave = None
                # normalization, entirely off the PE queue: the dense-ones
                # sums matmul already left den broadcast across partitions
                rec = finp.tile([128, QT], f32, tag="rec")
                nc.vector.reciprocal_approx_fast(rec, smp)
                nc.vector.tensor_mul(ot_s[:, h, q0:q0 + QT], otp, rec)

        def outproj(qt):
            q0 = qt * QT
            for sc in range(QT // 128):
                s0 = q0 + sc * 128
                for et in range(E // QT):
                    yp = psO.tile([128, QT], f32, tag="yp")
                    for h in range(M):
                        nc.tensor.matmul(yp, lhsT=ot_s[:, h, s0:s0 + 128],
                                         rhs=wo_s[:, h, et * QT:(et + 1) * QT],
                                         start=(h == 0), stop=(h == M - 1))
                    yst = ystp.tile([128, QT], bf, tag="yst")
                    nc.scalar.copy(yst, yp)
                    nc.sync.dma_start(
                        out=out_d[s0:s0 + 128, et * QT:(et + 1) * QT], in_=yst)

        # ---- schedule: pipeline outproj(qt) behind proj(qt+1) ----
        kproj(0); vproj(0); qproj(0); attn(0)
        for qt in range(1, NQT):
            kproj(qt); vproj(qt); qproj(qt)
            outproj(qt - 1)
            attn(qt)
        outproj(NQT - 1)

    nc.compile()
    return nc


def get_module():
    if "nc" not in _CACHE:
        _CACHE["nc"] = _build_module()
    return _CACHE["nc"]


def host_prep(x, Wq, Wk, Wv):
    """Build the per-core device input arrays (bf16)."""
    bf = ml_dtypes.bfloat16
    x = np.ascontiguousarray(np.asarray(x, np.float32))

    def perm_cols(W):
        W = np.asarray(W, np.float32).copy()
        for h0 in range(0, W.shape[1], D):
            blk = W[:, h0:h0 + D]
            W[:, h0:h0 + D] = np.concatenate([blk[:, ::2], blk[:, 1::2]], 1)
        return W

    Wq_p = perm_cols(Wq).astype(bf)
    Wk_p = perm_cols(Wk).astype(bf)
    Wv_b = np.asarray(Wv, np.float32).astype(bf)

    inv = 1000.0 ** (-2.0 * np.arange(D // 2, dtype=np.float32) / D)
    ang = np.arange(S, dtype=np.float32)[:, None] * inv[None, :]
    cos_e = np.cos(ang).T
    sin_e = np.sin(ang).T
    cos_t = np.ascontiguousarray(np.concatenate([cos_e, cos_e], 0).astype(bf))
    # signed sin table: rows 0:64 = -sin (for the "even - odd*sin" half),
    # rows 64:128 = +sin (see kernel rope())
    sin_t = np.ascontiguousarray(np.concatenate([-sin_e, sin_e], 0).astype(bf))

    j = np.arange(KC)[None, :]
    p = np.arange(KC)[:, None]
    mask = np.ascontiguousarray((j - p >= 0).astype(bf))

    xT_b = [np.ascontiguousarray(x[b].T).astype(bf) for b in range(B)]
    return xT_b, Wq_p, Wk_p, Wv_b, cos_t, sin_t, mask


def _ensure_ntff_hook():
    """The agent image's `antenv` lacks `axon_hooks`, so trn_boot silently
    skipped registering the NTFF profile hook. Recreate the registry module
    and register the ctypes-based hook so trace=True works under axon."""
    import sys
    import types
    try:
        from antenv import axon_hooks  # noqa: F401
        return
    except ImportError:
        pass
    import antenv
    mod = types.ModuleType("antenv.axon_hooks")
    _h = [None]
    mod.set_axon_ntff_profile_hook = lambda h: _h.__setitem__(0, h)
    mod.get_axon_ntff_profile_hook = lambda: _h[0]
    sys.modules["antenv.axon_hooks"] = mod
    antenv.axon_hooks = mod
    try:
        from trn_agent_boot.trn_boot import _ntff_profile_via_ctypes
        hook = _ntff_profile_via_ctypes("/opt/axon/libaxon_pjrt.so")
        mod.set_axon_ntff_profile_hook(hook)
    except Exception as e:  # degrade to no-trace
        print("ntff hook registration failed:", e)


def run(inputs, trace=False, trace_cores=None):
    from concourse import bass_utils
    if trace:
        _ensure_ntff_hook()

    x = np.asarray(inputs["x"], np.float32)
    Wo = np.asarray(inputs["Wo"], np.float32)
    bo = np.asarray(inputs["bo"], np.float32)
    bf = ml_dtypes.bfloat16

    xT_b, Wq_p, Wk_p, Wv_b, cos_t, sin_t, mask = host_prep(
        x, inputs["Wq"], inputs["Wk"], inputs["Wv"])

    in_maps = []
    for core in range(8):
        b, g = divmod(core, 4)
        in_maps.append(dict(
            xT=xT_b[b],
            wq=np.ascontiguousarray(Wq_p[:, g * DQ:(g + 1) * DQ]),
            wk=np.ascontiguousarray(Wk_p[:, g * D:(g + 1) * D]),
            wv=np.ascontiguousarray(Wv_b[:, g * D:(g + 1) * D]),
            wo=np.ascontiguousarray(Wo[g * DQ:(g + 1) * DQ, :].astype(bf)),
            cos_t=cos_t, sin_t=sin_t, mask=mask,
        ))

    nc = get_module()
    kw = {}
    if trace:
        kw = dict(trace=True,
                  trace_cores=trace_cores if trace_cores is not None else [0])
    res = bass_utils.run_bass_kernel_spmd(nc, in_maps, core_ids=list(range(8)), **kw)

    out = np.empty((B, S, E), np.float32)
    for b in range(B):
        acc = np.zeros((S, E), np.float32)
        for g in range(G):
            acc += np.asarray(res.results[4 * b + g]["out"], dtype=np.float32)
        out[b] = acc + bo[None, :]
    return out, res


def kernel(**inputs):
    out, _ = run(inputs, trace=False)
    return out


# revision 26
# speedup vs baseline: 1.0348x; 1.0324x over previous
"""GQA forward (B=2,S=2048,E=2048,H=16,G=4,D=128) on 8 TRN2 NeuronCores.

Sharding: core = (batch b, kv-group g), b=core//4, g=core%4. Each core
computes its group's 4 query heads end-to-end (QKV proj + RoPE + causal
attention + partial out-projection over its 512 Wo rows). Host sums the 4
partial outputs per batch and adds the bias (the unshard step of the
row-parallel out-projection).

Device dataflow (bf16 matmuls, fp32 PSUM):
  - x^T is staged as 4 column-block tiles and DMA'd in priority order
    (wk/cos/sin/mask, xtb0, wq, wv, xtb1, xtb2, wo, xtb3) so the K/Q/V
    projections start ~10us in, overlapping the rest of the input load.
  - Wq/Wk columns are permuted evens-first within each 128-col head block so
    RoPE's pair rotation becomes a fixed +-64-partition offset in the
    [d, seq] layout: out[0:64] = u*cos[0:64] - (u*sin)[64:128], etc.
  - Scores are computed transposed, ST[k,q] = K'^T-chunk x Q'-tile, so the
    exp'd tile PT[k,q] is directly the moving operand of the P@V matmul
    (OT[d,q] += V_chunk.T @ PT) -- no on-chip transpose of P.
  - Causal triangle-skip: on diagonal k-chunks (dm=0..3) the ST/exp/PV/sums
    work is restricted to q-columns >= dm*128; only the first 128-col
    subblock is triangular and gets a single [128,128] 0/1 mask multiply.
  - Softmax has no max-subtraction (scaled scores are bounded ~ +-17 here);
    denominators come from a ones-column matmul accumulated alongside PV.
  - Normalization runs entirely off the PE queue: reciprocal_approx_fast
    (DVE) -> partition_broadcast (GpSimd) -> tensor_mul (DVE), so the
    tensor engine flows from one head's chunks straight into the next.
  - Schedule interleaves outproj(qt) behind kproj/qproj/vproj(qt+1) to keep
    the PE busy while the last head's normalization chain completes.
"""

import numpy as np
import ml_dtypes

B, S, E = 2, 2048, 2048
H, G = 16, 4
D = E // H            # 128 head dim
M = H // G            # 4 heads per group
DQ = M * D            # 512 per-core Q columns
QT = 512              # q tile (moving dim)
KC = 128              # k chunk (partition dim of ST)
NEC = E // 128        # 16 contraction chunks
NQT = S // QT         # 4 q tiles
SCALE = 1.0 / float(np.sqrt(D))

_CACHE = {}


def _build_module():
    import concourse.tile as tile
    import concourse.mybir as mybir
    from concourse import bacc
    from contextlib import ExitStack

    bf = mybir.dt.bfloat16
    f32 = mybir.dt.float32
    AF = mybir.ActivationFunctionType

    nc = bacc.Bacc("TRN2", target_bir_lowering=False, debug=False)

    xT = nc.dram_tensor("xT", [E, S], bf, kind="ExternalInput").ap()
    wq = nc.dram_tensor("wq", [E, DQ], bf, kind="ExternalInput").ap()
    wk = nc.dram_tensor("wk", [E, D], bf, kind="ExternalInput").ap()
    wv = nc.dram_tensor("wv", [E, D], bf, kind="ExternalInput").ap()
    wo = nc.dram_tensor("wo", [DQ, E], bf, kind="ExternalInput").ap()
    cos_d = nc.dram_tensor("cos_t", [D, S], bf, kind="ExternalInput").ap()
    sin_d = nc.dram_tensor("sin_t", [D, S], bf, kind="ExternalInput").ap()
    mask_d = nc.dram_tensor("mask", [KC, KC], bf, kind="ExternalInput").ap()
    out_d = nc.dram_tensor("out", [S, E], bf, kind="ExternalOutput").ap()

    with tile.TileContext(nc) as tc, ExitStack() as ctx:
        singles = ctx.enter_context(tc.tile_pool(name="singles", bufs=1))
        ropep = ctx.enter_context(tc.tile_pool(name="ropep", bufs=2))
        ptp = ctx.enter_context(tc.tile_pool(name="ptp", bufs=6))
        finp = ctx.enter_context(tc.tile_pool(name="finp", bufs=2))
        ystp = ctx.enter_context(tc.tile_pool(name="ystp", bufs=4))
        psA = ctx.enter_context(tc.tile_pool(name="psA", bufs=3, space="PSUM"))
        psB = ctx.enter_context(tc.tile_pool(name="psB", bufs=2, space="PSUM"))
        psS = ctx.enter_context(tc.tile_pool(name="psS", bufs=1, space="PSUM"))
        psO = ctx.enter_context(tc.tile_pool(name="psO", bufs=2, space="PSUM"))

        # ---- resident SBUF tensors ----
        xtb = [singles.tile([128, NEC, QT], bf, name=f"xtb{t}", tag=f"xt{t}")
               for t in range(NQT)]
        wq_s = singles.tile([128, NEC, DQ], bf, tag="wq")
        wk_s = singles.tile([128, NEC, D], bf, tag="wk")
        wv_s = singles.tile([128, NEC, D], bf, tag="wv")
        wo_s = singles.tile([128, M, E], bf, tag="wo")          # [o%128, head, e]
        cos_s = singles.tile([128, S], bf, tag="cos")
        sin_s = singles.tile([128, S], bf, tag="sin")
        mask_s = singles.tile([128, KC], bf, tag="mask")
        kt_s = singles.tile([128, S], bf, tag="kt")             # roped K^T [d, k]
        v_s = singles.tile([128, NEC, D], bf, tag="v")          # V natural [k%128, k//128, d]
        qt_s = singles.tile([128, M, S], bf, tag="qt")          # roped Q^T [d, h, q]
        ot_s = singles.tile([128, M, S], bf, tag="ot")          # normalized O^T [d, h, q]
        ones_sq = singles.tile([128, 128], bf, tag="ones_sq")   # dense sums lhsT
        wz = singles.tile([128, QT], bf, tag="wz")              # warmup rhs

        nc.vector.memset(ones_sq, 1.0)
        nc.vector.memset(wz, 0.0)

        # ---- input DMAs: FEW large dma_starts (each fans out across HW-DGE
        # queues); issue order = priority. xtb0 goes in 4 pieces so kproj(0)'s
        # psum chain can chase the arriving ec-chunks. ----
        def dma_xtb(t, pieces=1):
            n = NEC // pieces
            for j in range(pieces):
                nc.sync.dma_start(
                    out=xtb[t][:, j * n:(j + 1) * n, :],
                    in_=xT[j * n * 128:(j + 1) * n * 128,
                           t * QT:(t + 1) * QT].rearrange("(n p) s -> p n s", p=128))

        nc.sync.dma_start(out=wk_s, in_=wk.rearrange("(n p) d -> p n d", p=128))
        nc.sync.dma_start(out=cos_s, in_=cos_d)
        nc.sync.dma_start(out=sin_s, in_=sin_d)
        dma_xtb(0, pieces=2)
        nc.sync.dma_start(out=mask_s, in_=mask_d)
        nc.sync.dma_start(out=wv_s, in_=wv.rearrange("(n p) d -> p n d", p=128))
        for h in range(M):
            nc.sync.dma_start(
                out=wq_s[:, :, h * D:(h + 1) * D],
                in_=wq[:, h * D:(h + 1) * D].rearrange("(n p) d -> p n d", p=128))
        dma_xtb(1)
        nc.sync.dma_start(out=wo_s, in_=wo.rearrange("(h p) e -> p h e", p=128))
        dma_xtb(2)
        dma_xtb(3)

        # ---- PE warmup: run dummy matmuls while the input DMAs stream so
        # the p-state ramp completes before kproj(0)'s first real matmul ----
        for i in range(44):
            wp = psB.tile([128, QT], f32, name="wp", tag="ot")
            nc.tensor.matmul(wp, lhsT=ones_sq, rhs=wz, start=True, stop=True)

        def rope(psum, pos0, n, out_bf):
            """psum [128, n] (evens-first d-layout) -> roped bf16 out_bf.

            sin_s is SIGNED: rows 0:64 hold -sin, rows 64:128 hold +sin, so
            out = psum*cos + swap_halves(psum)*sin_s. The half-swap happens in
            the PSUM-side read of the multiply (mixed PSUM+SBUF operands may
            have different base partitions; SBUF+SBUF may not)."""
            mc = ropep.tile([128, QT], f32, tag="mc")
            ms = ropep.tile([128, QT], f32, tag="ms")
            nc.vector.tensor_mul(mc[:, :n], psum, cos_s[:, pos0:pos0 + n])
            nc.vector.tensor_mul(ms[0:64, :n], psum[64:128], sin_s[0:64, pos0:pos0 + n])
            nc.vector.tensor_mul(ms[64:128, :n], psum[0:64], sin_s[64:128, pos0:pos0 + n])
            nc.vector.tensor_add(out_bf, mc[:, :n], ms[:, :n])

        def kproj(t):
            k0 = t * QT
            kp = psA.tile([128, QT], f32, tag="mm")
            for ec in range(NEC):
                nc.tensor.matmul(kp, lhsT=wk_s[:, ec, :], rhs=xtb[t][:, ec, :],
                                 start=(ec == 0), stop=(ec == NEC - 1))
            rope(kp, k0, QT, kt_s[:, k0:k0 + QT])

        def vproj(t):
            for j in range(QT // 128):
                kc = t * (QT // 128) + j
                vp = psA.tile([128, D], f32, tag="mm")
                for ec in range(NEC):
                    nc.tensor.matmul(vp, lhsT=xtb[t][:, ec, j * 128:(j + 1) * 128],
                                     rhs=wv_s[:, ec, :],
                                     start=(ec == 0), stop=(ec == NEC - 1))
                nc.scalar.copy(v_s[:, kc, :], vp)

        def qproj(qt):
            q0 = qt * QT
            for h in range(M):
                qp = psA.tile([128, QT], f32, tag="mm")
                for ec in range(NEC):
                    nc.tensor.matmul(qp, lhsT=wq_s[:, ec, h * D:(h + 1) * D],
                                     rhs=xtb[qt][:, ec, :],
                                     start=(ec == 0), stop=(ec == NEC - 1))
                rope(qp, q0, QT, qt_s[:, h, q0:q0 + QT])

        def attn(qt):
            q0 = qt * QT
            nkc = (q0 + QT) // KC
            fulls = qt * (QT // KC)     # below-diagonal chunks, even count
            for h in range(M):
                otp = psB.tile([128, QT], f32, tag="ot")
                smp = psS.tile([128, QT], f32, tag="sums")
                pt_prev = None
                for kc in range(nkc):
                    dm = kc - fulls
                    off = dm * 128 if dm > 0 else 0
                    stp = psA.tile([128, QT], f32, tag="mm")
                    nc.tensor.matmul(stp[:, off:], lhsT=kt_s[:, kc * KC:(kc + 1) * KC],
                                     rhs=qt_s[:, h, q0 + off:q0 + QT],
                                     start=True, stop=True)
                    pt = ptp.tile([128, QT], bf, tag="pt")
                    nc.scalar.activation(pt[:, off:], stp[:, off:], AF.Exp, scale=SCALE)
                    if dm >= 0:
                        nc.vector.tensor_mul(pt[:, off:off + 128],
                                             pt[:, off:off + 128], mask_s)
                    nc.tensor.matmul(otp[:, off:], lhsT=v_s[:, kc, :], rhs=pt[:, off:],
                                     start=(kc == 0), stop=(kc == nkc - 1))
                    # denominator: full chunks contribute as DVE-summed pairs
                    # (one ones-matmul per pair); diagonal chunks individually
                    if kc < fulls:
                        if kc % 2 == 0:
                            pt_prev = pt
                        else:
                            psum2 = ptp.tile([128, QT], bf, name="psum2", tag="pt")
                            nc.vector.tensor_add(psum2, pt_prev, pt)
                            nc.tensor.matmul(smp, lhsT=ones_sq, rhs=psum2,
                                             start=(kc == 1), stop=False)
                    else:
                        nc.tensor.matmul(smp[:, off:], lhsT=ones_sq, rhs=pt[:, off:],
                                         start=(kc == 0), stop=(kc == nkc - 1))
                # normalization, entirely off the PE queue: the dense-ones
                # sums matmul already left den broadcast across partitions
                rec = finp.tile([128, QT], f32, tag="rec")
                nc.vector.reciprocal_approx_fast(rec, smp)
                nc.vector.tensor_mul(ot_s[:, h, q0:q0 + QT], otp, rec)

        def outproj(qt):
            q0 = qt * QT
            for sc in range(QT // 128):
                s0 = q0 + sc * 128
                for et in range(E // QT):
                    yp = psO.tile([128, QT], f32, tag="yp")
                    for h in range(M):
                        nc.tensor.matmul(yp, lhsT=ot_s[:, h, s0:s0 + 128],
                                         rhs=wo_s[:, h, et * QT:(et + 1) * QT],
                                         start=(h == 0), stop=(h == M - 1))
                    yst = ystp.tile([128, QT], bf, tag="yst")
                    nc.scalar.copy(yst, yp)
                    nc.sync.dma_start(
                        out=out_d[s0:s0 + 128, et * QT:(et + 1) * QT], in_=yst)

        # ---- schedule: pipeline outproj(qt) behind proj(qt+1) ----
        kproj(0); vproj(0); qproj(0); attn(0)
        for qt in range(1, NQT):
            kproj(qt); vproj(qt); qproj(qt)
            outproj(qt - 1)
            attn(qt)
        outproj(NQT - 1)

    nc.compile()
    return nc


def get_module():
    if "nc" not in _CACHE:
        _CACHE["nc"] = _build_module()
    return _CACHE["nc"]


def host_prep(x, Wq, Wk, Wv):
    """Build the per-core device input arrays (bf16)."""
    bf = ml_dtypes.bfloat16
    x = np.ascontiguousarray(np.asarray(x, np.float32))

    def perm_cols(W):
        W = np.asarray(W, np.float32).copy()
        for h0 in range(0, W.shape[1], D):
            blk = W[:, h0:h0 + D]
            W[:, h0:h0 + D] = np.concatenate([blk[:, ::2], blk[:, 1::2]], 1)
        return W

    Wq_p = perm_cols(Wq).astype(bf)
    Wk_p = perm_cols(Wk).astype(bf)
    Wv_b = np.asarray(Wv, np.float32).astype(bf)

    inv = 1000.0 ** (-2.0 * np.arange(D // 2, dtype=np.float32) / D)
    ang = np.arange(S, dtype=np.float32)[:, None] * inv[None, :]
    cos_e = np.cos(ang).T
    sin_e = np.sin(ang).T
    cos_t = np.ascontiguousarray(np.concatenate([cos_e, cos_e], 0).astype(bf))
    # signed sin table: rows 0:64 = -sin (for the "even - odd*sin" half),
    # rows 64:128 = +sin (see kernel rope())
    sin_t = np.ascontiguousarray(np.concatenate([-sin_e, sin_e], 0).astype(bf))

    j = np.arange(KC)[None, :]
    p = np.arange(KC)[:, None]
    mask = np.ascontiguousarray((j - p >= 0).astype(bf))

    xT_b = [np.ascontiguousarray(x[b].T).astype(bf) for b in range(B)]
    return xT_b, Wq_p, Wk_p, Wv_b, cos_t, sin_t, mask


def _ensure_ntff_hook():
    """The agent image's `antenv` lacks `axon_hooks`, so trn_boot silently
    skipped registering the NTFF profile hook. Recreate the registry module
    and register the ctypes-based hook so trace=True works under axon."""
    import sys
    import types
    try:
        from antenv import axon_hooks  # noqa: F401
        return
    except ImportError:
        pass
    import antenv
    mod = types.ModuleType("antenv.axon_hooks")
    _h = [None]
    mod.set_axon_ntff_profile_hook = lambda h: _h.__setitem__(0, h)
    mod.get_axon_ntff_profile_hook = lambda: _h[0]
    sys.modules["antenv.axon_hooks"] = mod
    antenv.axon_hooks = mod
    try:
        from trn_agent_boot.trn_boot import _ntff_profile_via_ctypes
        hook = _ntff_profile_via_ctypes("/opt/axon/libaxon_pjrt.so")
        mod.set_axon_ntff_profile_hook(hook)
    except Exception as e:  # degrade to no-trace
        print("ntff hook registration failed:", e)


def run(inputs, trace=False, trace_cores=None):
    from concourse import bass_utils
    if trace:
        _ensure_ntff_hook()

    x = np.asarray(inputs["x"], np.float32)
    Wo = np.asarray(inputs["Wo"], np.float32)
    bo = np.asarray(inputs["bo"], np.float32)
    bf = ml_dtypes.bfloat16

    xT_b, Wq_p, Wk_p, Wv_b, cos_t, sin_t, mask = host_prep(
        x, inputs["Wq"], inputs["Wk"], inputs["Wv"])

    in_maps = []
    for core in range(8):
        b, g = divmod(core, 4)
        in_maps.append(dict(
            xT=xT_b[b],
            wq=np.ascontiguousarray(Wq_p[:, g * DQ:(g + 1) * DQ]),
            wk=np.ascontiguousarray(Wk_p[:, g * D:(g + 1) * D]),
            wv=np.ascontiguousarray(Wv_b[:, g * D:(g + 1) * D]),
            wo=np.ascontiguousarray(Wo[g * DQ:(g + 1) * DQ, :].astype(bf)),
            cos_t=cos_t, sin_t=sin_t, mask=mask,
        ))

    nc = get_module()
    kw = {}
    if trace:
        kw = dict(trace=True,
                  trace_cores=trace_cores if trace_cores is not None else [0])
    res = bass_utils.run_bass_kernel_spmd(nc, in_maps, core_ids=list(range(8)), **kw)

    out = np.empty((B, S, E), np.float32)
    for b in range(B):
        acc = np.zeros((S, E), np.float32)
        for g in range(G):
            acc += np.asarray(res.results[4 * b + g]["out"], dtype=np.float32)
        out[b] = acc + bo[None, :]
    return out, res


def kernel(**inputs):
    out, _ = run(inputs, trace=False)
    return out
